# revision 1
# baseline (speedup 1.0000x reference)
"""GCN (7-layer) + mean-pool Trainium2 kernel, 8-core SPMD.

Distribution: nodes (and their incident in-edges) sharded contiguously across
8 cores; 128x128 weights replicated; per layer each core builds its shard of
the feature table (dinv * h @ W, bf16), AllGathers the full table, then
aggregates its local destinations with dma_gather + one-hot segment-sum
matmuls accumulated in PSUM.  Mean-pool is a final matmul against a
host-built (1/cnt-scaled) graph-membership matrix; per-core partials are
summed on the host.
"""
import sys
import types

import numpy as np

import concourse.bacc as bacc
import concourse.mybir as mybir
import concourse.tile as tile
from concourse.bass_utils import run_bass_kernel_spmd
from concourse.masks import make_identity

# ---------------- problem constants (hardcoded per spec) ----------------
N = 100000
E = 1600000
F = 128
L = 7
G = 512
NC = 8
NLOC = N // NC              # 12500 real nodes per core
NPAD = 12800                # padded nodes per core (100 blocks of 128)
NBLKROWS = 25600            # gather source block rows (int16-safe < 32768)
NBLK = (NPAD * NC) // NBLKROWS  # 4
WIN = 512                   # dst window = one PSUM bank
NWIN = NPAD // WIN          # 25
NODE_CHUNKS = NPAD // 512   # 25 transform chunks
NBLOCK128 = NPAD // 128     # 100

F32 = mybir.dt.float32
BF16 = mybir.dt.bfloat16
I16 = mybir.dt.int16

_SO_PATH = "/opt/axon/libaxon_pjrt.so"


def _install_profile_hook():
    if "antenv.axon_hooks" in sys.modules:
        return
    try:
        from trn_agent_boot.trn_boot import _ntff_profile_via_ctypes
    except Exception:
        return
    hook = _ntff_profile_via_ctypes(_SO_PATH)
    mod = types.ModuleType("antenv.axon_hooks")
    mod.get_axon_ntff_profile_hook = lambda: hook
    mod.set_axon_ntff_profile_hook = lambda h: None
    sys.modules["antenv.axon_hooks"] = mod
    try:
        import antenv

        antenv.axon_hooks = mod
    except Exception:
        pass


def _wrap_idx(idx):
    """[n] int -> dma_gather wrapped layout [128, n//16] int16 (replicated
    into each 16-partition group)."""
    n = idx.shape[0]
    assert n % 16 == 0
    w = np.zeros((128, n // 16), np.int16)
    blk = idx.reshape(n // 16, 16).T.astype(np.int16)  # [16, n//16]
    for g in range(8):
        w[g * 16 : (g + 1) * 16, :] = blk
    return w


def _preprocess(x, edge_index, batch):
    """Host-side graph preprocessing -> per-core input maps + metadata."""
    src = np.asarray(edge_index[0], dtype=np.int64)
    dst = np.asarray(edge_index[1], dtype=np.int64)
    loops = np.arange(N, dtype=np.int64)
    src = np.concatenate([src, loops])
    dst = np.concatenate([dst, loops])
    deg = np.bincount(dst, minlength=N).astype(np.float32)  # >=1 (self loop)
    dinv = 1.0 / np.sqrt(deg)

    # global node id -> padded table row
    def table_row(n):
        return (n // NLOC) * NPAD + (n % NLOC)

    src_row = table_row(src)

    core_of = dst // NLOC
    dstloc = dst % NLOC

    # per-core cell membership
    # cells: (w, b) with w = dstloc//WIN, b = src_row//NBLKROWS
    w_of = dstloc // WIN
    b_of = src_row // NBLKROWS

    counts = np.zeros((NC, NWIN, NBLK), np.int64)
    for c in range(NC):
        m = core_of == c
        np.add.at(counts[c], (w_of[m], b_of[m]), 1)
    ntiles = np.maximum(1, -(-counts.max(axis=0) // 128))  # [NWIN, NBLK]
    tot_tiles = int(ntiles.sum())

    # per-core streams
    idxw_all = np.zeros((NC, 128, tot_tiles * 8), np.int16)
    dstoff_all = np.full((NC, 128, tot_tiles), 600.0, np.float32)
    for c in range(NC):
        m = core_of == c
        sr = src_row[m]
        dl = dstloc[m]
        wv = w_of[m]
        bv = b_of[m]
        order = np.lexsort((dl, bv, wv))
        sr, dl, wv, bv = sr[order], dl[order], wv[order], bv[order]
        pos = 0
        tbase = 0
        for w in range(NWIN):
            for b in range(NBLK):
                nt = int(ntiles[w, b])
                cnt = int(counts[c, w, b])
                sl = slice(pos, pos + cnt)
                cap = nt * 128
                idx_local = np.zeros(cap, np.int64)
                idx_local[:cnt] = sr[sl] - b * NBLKROWS
                doff = np.full(cap, 600.0, np.float32)
                doff[:cnt] = (dl[sl] - w * WIN).astype(np.float32)
                idxw_all[c, :, tbase * 8 : (tbase + nt) * 8] = _wrap_idx(idx_local)
                dstoff_all[c, :, tbase : tbase + nt] = doff.reshape(nt, 128).T
                pos += cnt
                tbase += nt
        assert pos == int(m.sum())

    # dinv streams
    dinv_pad = np.ones(NPAD, np.float32)
    dinv_rep = np.zeros((NC, 128, NPAD), np.float32)
    dinv_nodecol = np.ones((NC, 128, NBLOCK128), np.float32)
    for c in range(NC):
        dl = dinv[c * NLOC : (c + 1) * NLOC]
        dp = dinv_pad.copy()
        dp[:NLOC] = dl
        dinv_rep[c] = np.broadcast_to(dp, (128, NPAD))
        dinv_nodecol[c] = dp.reshape(NBLOCK128, 128).T

    # pooling matrices: S_pool[c, blk][n, j] = 1/cnt[g0_c + j] if
    # batch[node]==g0_c+j else 0 (zero rows for pad nodes)
    batch = np.asarray(batch, dtype=np.int64)
    cnt = np.bincount(batch, minlength=G).astype(np.float32)
    cnt = np.maximum(cnt, 1.0)
    g0 = np.zeros(NC, np.int64)
    s_pool = np.zeros((NC, NBLOCK128, 128, 128), np.float32)
    for c in range(NC):
        bl = batch[c * NLOC : (c + 1) * NLOC]
        g0[c] = bl[0]
        j = bl - g0[c]
        assert j.max() < 128, "graph span exceeds 128 on a core"
        val = (1.0 / cnt[bl]).astype(np.float32)
        n_loc = np.arange(NLOC)
        sp = np.zeros((NPAD, 128), np.float32)
        sp[n_loc, j] = val
        s_pool[c] = sp.reshape(NBLOCK128, 128, 128)

    meta = {
        "ntiles": ntiles,
        "g0": g0,
    }
    return idxw_all, dstoff_all, dinv_rep, dinv_nodecol, s_pool, meta


def _build(ntiles):
    """Build the SPMD Bacc graph. `ntiles` [NWIN, NBLK] is compile-time."""
    nc = bacc.Bacc()
    tot_tiles = int(ntiles.sum())

    xT_p = nc.declare_dram_parameter("xT", [128, NPAD], F32, isOutput=False)
    W_p = nc.declare_dram_parameter("Wstack", [L, 128, 128], F32, isOutput=False)
    b_p = nc.declare_dram_parameter("bstack", [L, 128], F32, isOutput=False)
    dinvrep_p = nc.declare_dram_parameter("dinv_rep", [128, NPAD], F32, isOutput=False)
    dinvcol_p = nc.declare_dram_parameter(
        "dinv_nodecol", [128, NBLOCK128], F32, isOutput=False
    )
    iota_p = nc.declare_dram_parameter("iota512", [128, WIN], F32, isOutput=False)
    idxw_p = nc.declare_dram_parameter(
        "idxw", [128, tot_tiles * 8], I16, isOutput=False
    )
    dstoff_p = nc.declare_dram_parameter(
        "dstoff", [128, tot_tiles], F32, isOutput=False
    )
    spool_p = nc.declare_dram_parameter(
        "s_pool", [NBLOCK128, 128, 128], F32, isOutput=False
    )
    out_p = nc.declare_dram_parameter("out", [128, 128], F32, isOutput=True)

    # internal DRAM
    shard = nc.dram_tensor("shard_bf16", [NPAD, F], BF16)
    table = nc.dram_tensor("table_bf16", [NPAD * NC, F], BF16, addr_space="Shared")
    tblk = [
        nc.dram_tensor(f"tblk{b}", [NBLKROWS, F], BF16) for b in range(NBLK)
    ]

    with tile.TileContext(nc) as tc:
        with (
            tc.tile_pool(name="big", bufs=1) as big,
            tc.tile_pool(name="consts", bufs=1) as consts,
            tc.tile_pool(name="work", bufs=3) as work,
            tc.tile_pool(name="gath", bufs=2) as gath,
            tc.tile_pool(name="sseg", bufs=4) as sseg,
            tc.tile_pool(name="ps", bufs=1, space="PSUM") as ps,
            tc.tile_pool(name="psagg", bufs=2, space="PSUM") as psagg,
        ):
            h = big.tile([128, NPAD], F32)          # [feat, node]
            dinv_rep = big.tile([128, NPAD], F32)
            dinv_col = consts.tile([128, NBLOCK128], F32)
            iota = consts.tile([128, WIN], F32)
            Wt = consts.tile([128, L, 128], F32)
            bias = consts.tile([128, L], F32)

            ident = consts.tile([128, 128], F32)
            make_identity(nc, ident[:])
            nc.sync.dma_start(out=h[:], in_=xT_p[:])
            nc.sync.dma_start(out=dinv_rep[:], in_=dinvrep_p[:])
            nc.sync.dma_start(out=dinv_col[:], in_=dinvcol_p[:])
            nc.sync.dma_start(out=iota[:], in_=iota_p[:])
            nc.sync.dma_start(out=Wt[:], in_=W_p[:].rearrange("l a b -> a l b"))
            # bias [L,128] -> [128 partitions, L]
            nc.sync.dma_start(
                out=bias[:], in_=b_p[:].rearrange("l f -> f l")
            )

            for layer in range(L):
                # ---- transform: h' = W^T @ h ; table row n = dinv[n]*h'[:,n]
                for ch in range(NODE_CHUNKS):
                    hp = ps.tile([128, 512], F32, name=f"hp_{layer}_{ch}", tag="hp")
                    nc.tensor.matmul(
                        out=hp[:],
                        lhsT=Wt[:, layer, :],
                        rhs=h[:, ch * 512 : (ch + 1) * 512],
                        start=True,
                        stop=True,
                    )
                    stg = work.tile([128, 512], F32, name=f"stg_{layer}_{ch}", tag="stg")
                    nc.vector.tensor_copy(out=stg[:], in_=hp[:])
                    tstage = work.tile(
                        [128, 4, 128], BF16, name=f"tstage_{layer}_{ch}", tag="tstage"
                    )
                    for j in range(4):
                        blk128 = ch * 4 + j
                        tp = ps.tile(
                            [128, 128], F32, name=f"tp_{layer}_{ch}_{j}", tag="tp"
                        )
                        nc.tensor.transpose(
                            out=tp[:],
                            in_=stg[:, j * 128 : (j + 1) * 128],
                            identity=ident[:],
                        )
                        nc.vector.tensor_scalar_mul(
                            tstage[:, j, :], tp[:], dinv_col[:, blk128 : blk128 + 1]
                        )
                    for j in range(4):
                        r0 = ch * 512 + j * 128
                        nc.sync.dma_start(
                            out=shard[r0 : r0 + 128, :], in_=tstage[:, j, :]
                        )
                nc.gpsimd.collective_compute(
                    "AllGather",
                    mybir.AluOpType.bypass,
                    replica_groups=[list(range(NC))],
                    ins=[shard[:]],
                    outs=[table[:]],
                )
                # dma_gather ignores AP offsets on its source: copy each
                # 25600-row block to a zero-offset tensor
                CPR = 6400  # rows per copy chunk (=128*50)
                for b in range(NBLK):
                    for r0 in range(0, NBLKROWS, CPR):
                        tcp = work.tile(
                            [128, CPR * F // 128],
                            BF16,
                            name=f"tcp_{layer}_{b}_{r0}",
                            tag="tcp",
                        )
                        a0 = b * NBLKROWS + r0
                        nc.sync.dma_start(
                            out=tcp[:],
                            in_=table[a0 : a0 + CPR, :].rearrange(
                                "(p r) d -> p (r d)", p=128
                            ),
                        )
                        nc.sync.dma_start(
                            out=tblk[b][r0 : r0 + CPR, :].rearrange(
                                "(p r) d -> p (r d)", p=128
                            ),
                            in_=tcp[:],
                        )

                # ---- aggregate into h (overwritten window by window)
                tbase = 0
                for w in range(NWIN):
                    agg = psagg.tile([128, WIN], F32, name=f"agg_{layer}_{w}", tag="agg")
                    n_in_win = int(ntiles[w].sum())
                    ti = 0
                    for b in range(NBLK):
                        nt = int(ntiles[w, b])
                        msg = gath.tile(
                            [128, nt * 128 // 128, 128],
                            BF16,
                            name=f"msg_{layer}_{w}_{b}",
                            tag="msg",
                            padded_shape=[128, 24, 128],
                        )
                        idxs = gath.tile(
                            [128, nt * 8],
                            I16,
                            name=f"idx_{layer}_{w}_{b}",
                            tag="idx",
                            padded_shape=[128, 24 * 8],
                        )
                        nc.sync.dma_start(
                            out=idxs[:],
                            in_=idxw_p[:, tbase * 8 : (tbase + nt) * 8],
                        )
                        for k0 in range(0, nt, 16):
                            kn = min(16, nt - k0)
                            nc.gpsimd.dma_gather(
                                out_ap=msg[:, k0 : k0 + kn, :],
                                in_ap=tblk[b][:],
                                idxs_ap=idxs[:, k0 * 8 : (k0 + kn) * 8],
                                num_idxs=kn * 128,
                                num_idxs_reg=kn * 128,
                                elem_size=F,
                                single_packet=False,
                            )
                        doffs = gath.tile(
                            [128, nt],
                            F32,
                            name=f"dof_{layer}_{w}_{b}",
                            tag="dof",
                            padded_shape=[128, 24],
                        )
                        nc.sync.dma_start(
                            out=doffs[:], in_=dstoff_p[:, tbase : tbase + nt]
                        )
                        for t in range(nt):
                            S = sseg.tile(
                                [128, WIN], BF16, name=f"S_{layer}_{w}_{b}_{t}", tag="S"
                            )
                            nc.vector.tensor_tensor(
                                out=S[:],
                                in0=doffs[:, t : t + 1].to_broadcast([128, WIN]),
                                in1=iota[:],
                                op=mybir.AluOpType.is_equal,
                            )
                            nc.tensor.matmul(
                                out=agg[:],
                                lhsT=msg[:, t, :],
                                rhs=S[:],
                                start=(ti == 0),
                                stop=(ti == n_in_win - 1),
                            )
                            ti += 1
                        tbase += nt
                    # copy-out: h[:, win] = relu(dinv*agg + bias)
                    tmp = work.tile([128, WIN], F32, name=f"tmp_{layer}_{w}", tag="tmp")
                    nc.vector.tensor_mul(
                        out=tmp[:],
                        in0=agg[:],
                        in1=dinv_rep[:, w * WIN : (w + 1) * WIN],
                    )
                    nc.scalar.activation(
                        out=h[:, w * WIN : (w + 1) * WIN],
                        in_=tmp[:],
                        func=mybir.ActivationFunctionType.Relu,
                        bias=bias[:, layer : layer + 1],
                    )

            # ---- mean pool: out[j, f] = sum_n S_pool[n, j] * h[f, n]
            pool_ps = ps.tile([128, 128], F32)
            for blk in range(NBLOCK128):
                sp = work.tile([128, 128], F32, name=f"sp_{blk}", tag="sp")
                nc.sync.dma_start(out=sp[:], in_=spool_p[blk])
                tp2 = ps.tile([128, 128], F32, name=f"tp2_{blk}", tag="tp2")
                nc.tensor.transpose(
                    out=tp2[:],
                    in_=h[:, blk * 128 : (blk + 1) * 128],
                    identity=ident[:],
                )
                hT = work.tile([128, 128], F32, name=f"hT_{blk}", tag="hT")
                nc.vector.tensor_copy(out=hT[:], in_=tp2[:])
                nc.tensor.matmul(
                    out=pool_ps[:],
                    lhsT=sp[:],
                    rhs=hT[:],
                    start=(blk == 0),
                    stop=(blk == NBLOCK128 - 1),
                )
            ores = work.tile([128, 128], F32)
            nc.vector.tensor_copy(out=ores[:], in_=pool_ps[:])
            nc.sync.dma_start(out=out_p[:], in_=ores[:])

    nc.finalize()
    return nc


def kernel(x, edge_index, batch, W0, Wh, b):
    x = np.asarray(x, dtype=np.float32)
    W0 = np.asarray(W0, dtype=np.float32)
    Wh = np.asarray(Wh, dtype=np.float32)
    b = np.asarray(b, dtype=np.float32)

    idxw, dstoff, dinv_rep, dinv_col, s_pool, meta = _preprocess(
        x, edge_index, batch
    )
    ntiles = meta["ntiles"]
    g0 = meta["g0"]

    Wstack = np.concatenate([W0[None], Wh], axis=0)  # [7,128,128]
    iota512 = np.broadcast_to(
        np.arange(WIN, dtype=np.float32), (128, WIN)
    ).copy()

    in_maps = []
    for c in range(NC):
        xT = np.zeros((128, NPAD), np.float32)
        xT[:, :NLOC] = x[c * NLOC : (c + 1) * NLOC].T
        in_maps.append(
            {
                "xT": xT,
                "Wstack": Wstack,
                "bstack": b,
                "dinv_rep": dinv_rep[c],
                "dinv_nodecol": dinv_col[c],
                "iota512": iota512,
                "idxw": idxw[c],
                "dstoff": dstoff[c],
                "s_pool": s_pool[c],
            }
        )

    nc = _build(ntiles)
    _install_profile_hook()
    import os

    trace = os.environ.get("GNN_TRACE", "0") == "1"
    res = run_bass_kernel_spmd(
        nc,
        in_maps,
        core_ids=list(range(NC)),
        trace=trace,
        tmpdir=os.environ.get("GNN_TRACE_DIR"),
    )
    if trace and res.exec_time_ns is not None:
        print(f"HW exec time: {res.exec_time_ns} ns")

    out = np.zeros((G, F), np.float32)
    for c in range(NC):
        oc = res.results[c]["out"]  # [128, 128]
        lo = int(g0[c])
        hi = min(G, lo + 128)
        out[lo:hi] += oc[: hi - lo]
    return out

